# revision 1
# baseline (speedup 1.0000x reference)
"""ComplexAttentionBlock on 8 Trainium2 NeuronCores.

Sharding: 8 cores = (4 batches) x (2 query-token halves). Each core computes
LayerNorm+QKV for all 1024 tokens of its batch (k/v need the full sequence),
but attention scores / attention output / proj / LN2 / MLP only for its 512
query tokens.  Tokens are permuted on host so every core's query tokens are
rows 0:511 of its input -> fully symmetric SPMD program, no collectives.

Layouts on chip:
  - activations feeding matmuls are feature-major ("T" = [feat, tok]) so the
    contraction dim always sits on SBUF partitions; v is token-major for the
    attention*V matmul.  Weights are pre-transposed on host to [in, out].
  - complex linear out_r = xr@Wr.T - xi@Wi.T uses a host-negated Wi copy so
    every output accumulates in a single PSUM bank.
  - softmax needs no max-subtraction (|attn|/8 is small so exp is safe); row
    sums come from a ones-column matmul; v and the attention output bounce
    through DRAM scratch to keep SBUF pool lifetimes strictly nested.
Matmuls run as float32r (TF32-like, 4x fp32 matmul throughput).
"""

import sys

sys.path.insert(0, "/opt/trn_rl_repo")

import numpy as np

import concourse.bacc as bacc
import concourse.tile as tile
from concourse import mybir
from concourse.bass_utils import run_bass_kernel_spmd
from concourse.masks import make_identity

F32 = mybir.dt.float32
F32R = mybir.dt.float32r
AF = mybir.ActivationFunctionType
OP = mybir.AluOpType

B, N, E, H, HD, MLP = 4, 1024, 768, 12, 64, 1536
NQ = N // 2          # query tokens per core
CE = E // 128        # 6 contraction chunks over E
CM = MLP // 128      # 12 chunks over MLP dim
EPS = 1e-6

NCORES = 8


def _mm(nc, ps, lhsT, rhs, start, stop):
    nc.tensor.matmul(ps, lhsT, rhs, start=start, stop=stop)


def build_program():
    nc = bacc.Bacc("TRN2", target_bir_lowering=False, debug=False)

    def din(name, shape, dt=F32R):
        return nc.dram_tensor(name, list(shape), dt, kind="ExternalInput").ap()

    xp = din("xp", [N, 2 * E], F32)
    qkv_wt3 = din("qkv_wt3", [E, 3, 3 * E])
    qkv_bcr = din("qkv_bcr", [128, 18], F32)
    qkv_bci = din("qkv_bci", [128, 18], F32)
    pj_wt3 = din("pj_wt3", [E, 3, E])
    pj_brr = din("pj_brr", [128, E], F32)
    pj_bri = din("pj_bri", [128, E], F32)
    m1_wt3 = din("m1_wt3", [E, 3, MLP])
    m1_bcr = din("m1_bcr", [128, CM], F32)
    m1_bci = din("m1_bci", [128, CM], F32)
    m2_wt3 = din("m2_wt3", [MLP, 3, E])
    m2_brr = din("m2_brr", [128, E], F32)
    m2_bri = din("m2_bri", [128, E], F32)
    g1r = din("g1r", [128, CE], F32)
    g1i = din("g1i", [128, CE], F32)
    b1r = din("b1r", [128, CE], F32)
    b1i = din("b1i", [128, CE], F32)
    g2r = din("g2r", [128, CE], F32)
    g2i = din("g2i", [128, CE], F32)
    b2r = din("b2r", [128, CE], F32)
    b2i = din("b2i", [128, CE], F32)
    ones_in = din("ones_in", [128, 1])
    qkv_vbr = din("qkv_vbr", [64, H], F32)
    qkv_vbi = din("qkv_vbi", [64, H], F32)

    out = nc.dram_tensor("out", [NQ, 2 * E], F32, kind="ExternalOutput").ap()

    # DRAM scratch bounces (keep SBUF pool lifetimes strictly nested)
    v_dr = nc.dram_tensor("v_dr", [N, E], F32R).ap()
    v_di = nc.dram_tensor("v_di", [N, E], F32R).ap()
    a_dr = nc.dram_tensor("a_dr", [E, NQ], F32R).ap()
    a_di = nc.dram_tensor("a_di", [E, NQ], F32R).ap()

    with tile.TileContext(nc) as tc, tc.tile_pool(name="const", bufs=1) as const:
        ident = const.tile([128, 128], F32)
        make_identity(nc, ident)
        ones_sb = const.tile([128, 1], F32R)
        nc.sync.dma_start(out=ones_sb, in_=ones_in)
        epsl = const.tile([128, 1], F32)
        nc.vector.memset(epsl, EPS)
        epsa = const.tile([128, 1], F32)
        nc.vector.memset(epsa, 1e-8)
        qbr_sb = const.tile([128, 18], F32)
        nc.sync.dma_start(out=qbr_sb, in_=qkv_bcr)
        qbi_sb = const.tile([128, 18], F32)
        nc.sync.dma_start(out=qbi_sb, in_=qkv_bci)
        m1br_sb = const.tile([128, CM], F32)
        nc.sync.dma_start(out=m1br_sb, in_=m1_bcr)
        m1bi_sb = const.tile([128, CM], F32)
        nc.sync.dma_start(out=m1bi_sb, in_=m1_bci)
        vbr_sb = const.tile([64, H], F32)
        nc.sync.dma_start(out=vbr_sb, in_=qkv_vbr)
        vbi_sb = const.tile([64, H], F32)
        nc.sync.dma_start(out=vbi_sb, in_=qkv_vbi)
        gb_sb = {}
        for nm, src in (("g1r", g1r), ("g1i", g1i), ("b1r", b1r), ("b1i", b1i),
                        ("g2r", g2r), ("g2i", g2i), ("b2r", b2r), ("b2i", b2i)):
            t = const.tile([128, CE], F32, name=nm, tag=nm)
            nc.sync.dma_start(out=t, in_=src)
            gb_sb[nm] = t

        def layer_norm(lnp, psT, xr, xi, gcr, gci, bcr, bci, dst_r, dst_i,
                       dst_col, tok128):
            """One 128-token LN tile -> gamma/beta-folded feature-major dst."""
            sq = lnp.tile([128, E], F32, tag="sq")
            nc.gpsimd.tensor_mul(sq, xr, xr)
            sqi = lnp.tile([128, E], F32, tag="sqi")
            nc.scalar.square(sqi, xi)
            ssum = lnp.tile([128, 2], F32, tag="ssum")
            nc.vector.scalar_tensor_tensor(sq, in0=sq, scalar=0.0, in1=sqi,
                                           op0=OP.add, op1=OP.add,
                                           accum_out=ssum[:, 0:1])
            mag = lnp.tile([128, E], F32, tag="mag")
            nc.scalar.activation(mag, sq, AF.Sqrt, bias=epsl,
                                 accum_out=ssum[:, 1:2])
            # mean/var from the free accumulations: mean = sum(mag)/E,
            # var = sum(mag^2)/E + EPS - mean^2  (mag^2 = m2 + EPS exactly)
            mv = lnp.tile([128, 4], F32, tag="mv")
            nc.vector.tensor_scalar_mul(mv[:, 0:1], ssum[:, 1:2], 1.0 / E)
            nc.vector.tensor_scalar(out=mv[:, 2:3], in0=ssum[:, 0:1],
                                    scalar1=1.0 / E, scalar2=EPS,
                                    op0=OP.mult, op1=OP.add)
            nc.vector.tensor_scalar(out=mv[:, 3:4], in0=mv[:, 0:1],
                                    scalar1=mv[:, 0:1], scalar2=None,
                                    op0=OP.mult)
            nc.vector.scalar_tensor_tensor(mv[:, 1:2], in0=mv[:, 2:3],
                                           scalar=0.0, in1=mv[:, 3:4],
                                           op0=OP.add, op1=OP.subtract)
            rstd = lnp.tile([128, 1], F32, tag="rstd")
            nc.scalar.activation(rstd, mv[:, 1:2], AF.Sqrt, bias=epsl)
            nc.vector.reciprocal(rstd, rstd)
            d = lnp.tile([128, E], F32, name="d", tag="sq")
            nc.scalar.activation(d, mag, AF.Copy, bias=EPS)
            nc.vector.reciprocal(d, d)
            scl = lnp.tile([128, E], F32, name="scl", tag="sqi")
            nc.vector.scalar_tensor_tensor(scl, in0=mag, scalar=mv[:, 0:1],
                                           in1=d, op0=OP.subtract, op1=OP.mult)
            x0r = lnp.tile([128, E], F32, tag="x0r")
            nc.vector.scalar_tensor_tensor(x0r, in0=xr, scalar=rstd, in1=scl,
                                           op0=OP.mult, op1=OP.mult)
            x0i = lnp.tile([128, E], F32, tag="x0i")
            nc.vector.scalar_tensor_tensor(x0i, in0=xi, scalar=rstd, in1=scl,
                                           op0=OP.mult, op1=OP.mult)
            for c in range(CE):
                for x0, dst, gc, bc, tg in ((x0r, dst_r, gcr, bcr, "pstr"),
                                            (x0i, dst_i, gci, bci, "psti")):
                    pst = psT.tile([128, 128], F32, name=tg, tag=tg)
                    nc.tensor.transpose(pst, x0[:, c * 128:(c + 1) * 128], ident)
                    nc.vector.tensor_scalar(
                        out=dst[:, dst_col(c):dst_col(c) + tok128],
                        in0=pst, scalar1=gc[:, c:c + 1], scalar2=bc[:, c:c + 1],
                        op0=OP.mult, op1=OP.add)

        with tc.tile_pool(name="qk", bufs=1) as qk_p:
            q_r = qk_p.tile([128, CE * NQ], F32R)
            q_i = qk_p.tile([128, CE * NQ], F32R)
            k_r = qk_p.tile([128, CE * N], F32R)
            k_i = qk_p.tile([128, CE * N], F32R)

            with tc.tile_pool(name="xnt", bufs=1) as xnt_p:
                xnt_r = xnt_p.tile([128, CE * N], F32R)
                xnt_i = xnt_p.tile([128, CE * N], F32R)

                # v-weight pool opens before LN1 so its DMAs prefetch
                # during stage A (bump-allocator release deps otherwise
                # gate them on the last LN instruction).
                # q/k weight pools open early: their first weight DMAs
                # prefetch during LN1/B.v instead of waiting for the
                # previous pools' release.
                with tc.tile_pool(name="wq", bufs=2) as wq, \
                     tc.tile_pool(name="wk", bufs=2) as wk:
                    with tc.tile_pool(name="wv", bufs=3) as wv, \
                         tc.tile_pool(name="vev", bufs=2) as vev, \
                         tc.tile_pool(name="psV", bufs=4, space="PSUM") as psV:
                        # ---------- stage A: LN1 + transpose to feature-major ----------
                        with tc.tile_pool(name="lnA", bufs=2) as lnp, \
                             tc.tile_pool(name="xin", bufs=3) as xin, \
                             tc.tile_pool(name="psA", bufs=2, space="PSUM") as psA:
                            for t in range(N // 128):
                                xt = xin.tile([128, 2 * E], F32, tag="xt")
                                nc.sync.dma_start(out=xt, in_=xp[t * 128:(t + 1) * 128, :])
                                xc = xt.rearrange("p (e c) -> p e c", c=2)
                                layer_norm(lnp, psA, xc[:, :, 0], xc[:, :, 1],
                                           gb_sb["g1r"], gb_sb["g1i"], gb_sb["b1r"], gb_sb["b1i"],
                                           xnt_r, xnt_i, lambda c, t=t: c * N + t * 128, 128)

                        # ---------- stage B.v: v (token-major) -> DRAM bounce ---
                        # psV opened before psA (2 banks) so the first v matmuls
                        # can run while LN1 is still streaming later token tiles.
                        for f0, fw in ((0, 512), (512, 256)):
                            for th2 in range(4):
                                ps = [[psV.tile([128, fw], F32, name="vps", tag="vps")
                                       for _ in range(2)] for _ in range(2)]
                                for ci in range(CE):
                                    w3 = wv.tile([128, 3, fw], F32R, tag="wv3")
                                    nc.sync.dma_start(out=w3, in_=qkv_wt3[ci * 128:(ci + 1) * 128, :, 2 * E + f0:2 * E + f0 + fw])
                                    wr, wi, win = w3[:, 0, :], w3[:, 1, :], w3[:, 2, :]
                                    for tjj in range(2):
                                        t = th2 * 2 + tjj
                                        xr_sl = xnt_r[:, ci * N + t * 128:ci * N + (t + 1) * 128]
                                        xi_sl = xnt_i[:, ci * N + t * 128:ci * N + (t + 1) * 128]
                                        _mm(nc, ps[tjj][0], xr_sl, wr, ci == 0, False)
                                        _mm(nc, ps[tjj][0], xi_sl, win, False, ci == CE - 1)
                                        _mm(nc, ps[tjj][1], xr_sl, wi, ci == 0, False)
                                        _mm(nc, ps[tjj][1], xi_sl, wr, False, ci == CE - 1)
                                for tjj in range(2):
                                    t = th2 * 2 + tjj
                                    for comp, dst in ((0, v_dr), (1, v_di)):
                                        ev = vev.tile([128, fw], F32R, name="vevt", tag="vevt")
                                        if comp == 0:
                                            nc.vector.tensor_copy(ev, ps[tjj][comp])
                                        else:
                                            nc.scalar.copy(ev, ps[tjj][comp])
                                        nc.sync.dma_start(out=dst[t * 128:(t + 1) * 128, f0:f0 + fw], in_=ev)

                    # ---- stage C pools open BEFORE B.q/B.k: the bump allocator
                    # adds release deps on reused zones, so opening these first
                    # lets attention overlap with the q/k matmul stages.
                    with tc.tile_pool(name="scp", bufs=2) as scp, \
                         tc.tile_pool(name="expp", bufs=2) as expp, \
                         tc.tile_pool(name="vst", bufs=2) as vst, \
                         tc.tile_pool(name="rsp", bufs=1) as rsp, \
                         tc.tile_pool(name="psS", bufs=2, space="PSUM") as psS:
                        # ---------- stage B.q (4 psum banks: cout pairs) --------
                        with tc.tile_pool(name="psQ", bufs=4, space="PSUM") as psQ:
                            for g in range(3):
                                ps = [[psQ.tile([128, NQ], F32, name="qps", tag="qps")
                                       for _ in range(2)] for _ in range(2)]
                                for ci in range(CE):
                                    w3 = wq.tile([128, 3, 256], F32R, tag="wq3")
                                    nc.sync.dma_start(out=w3, in_=qkv_wt3[ci * 128:(ci + 1) * 128, :, g * 256:(g + 1) * 256])
                                    wr, wi, win = w3[:, 0, :], w3[:, 1, :], w3[:, 2, :]
                                    xr_sl = xnt_r[:, ci * N:ci * N + NQ]
                                    xi_sl = xnt_i[:, ci * N:ci * N + NQ]
                                    for j in range(2):
                                        wsl = slice(j * 128, (j + 1) * 128)
                                        _mm(nc, ps[j][0], wr[:, wsl], xr_sl, ci == 0, False)
                                        _mm(nc, ps[j][0], win[:, wsl], xi_sl, False, ci == CE - 1)
                                        _mm(nc, ps[j][1], wi[:, wsl], xr_sl, ci == 0, False)
                                        _mm(nc, ps[j][1], wr[:, wsl], xi_sl, False, ci == CE - 1)
                                for j in range(2):
                                    co = 2 * g + j
                                    nc.scalar.activation(q_r[:, co * NQ:(co + 1) * NQ],
                                                         ps[j][0], AF.Identity,
                                                         bias=qbr_sb[:, co:co + 1])
                                    nc.scalar.activation(q_i[:, co * NQ:(co + 1) * NQ],
                                                         ps[j][1], AF.Identity,
                                                         bias=qbi_sb[:, co:co + 1])

                        # ---------- stage B.k (4 psum banks: single cout) -------
                        with tc.tile_pool(name="psK", bufs=4, space="PSUM") as psK:
                            for co in range(CE):
                                ps = [[psK.tile([128, NQ], F32, name="kps", tag="kps")
                                       for _ in range(2)] for _ in range(2)]
                                for ci in range(CE):
                                    w3 = wk.tile([128, 3, 128], F32R, tag="wk3")
                                    nc.sync.dma_start(out=w3, in_=qkv_wt3[ci * 128:(ci + 1) * 128, :, E + co * 128:E + (co + 1) * 128])
                                    wr, wi, win = w3[:, 0, :], w3[:, 1, :], w3[:, 2, :]
                                    for tk in range(2):
                                        xr_sl = xnt_r[:, ci * N + tk * NQ:ci * N + (tk + 1) * NQ]
                                        xi_sl = xnt_i[:, ci * N + tk * NQ:ci * N + (tk + 1) * NQ]
                                        _mm(nc, ps[tk][0], wr, xr_sl, ci == 0, False)
                                        _mm(nc, ps[tk][0], win, xi_sl, False, ci == CE - 1)
                                        _mm(nc, ps[tk][1], wi, xr_sl, ci == 0, False)
                                        _mm(nc, ps[tk][1], wr, xi_sl, False, ci == CE - 1)
                                for tk in range(2):
                                    dsl = slice(co * N + tk * NQ, co * N + (tk + 1) * NQ)
                                    nc.vector.tensor_scalar_add(
                                        k_r[:, dsl], ps[tk][0], qbr_sb[:, 6 + co:7 + co])
                                    nc.vector.tensor_scalar_add(
                                        k_i[:, dsl], ps[tk][1], qbi_sb[:, 6 + co:7 + co])

                        # ---------- stage C: attention -> DRAM bounce -----------
                        with tc.tile_pool(name="aev", bufs=2) as aevp, \
                             tc.tile_pool(name="psAt", bufs=1, space="PSUM") as psAt, \
                             tc.tile_pool(name="psSum", bufs=1, space="PSUM") as psSum:
                            v_dr_t = v_dr.rearrange("(kt p) e -> p kt e", p=128)
                            v_di_t = v_di.rearrange("(kt p) e -> p kt e", p=128)
                            for c in range(H // 2):
                                qineg = scp.tile([128, NQ], F32R, tag="qineg")
                                nc.vector.tensor_scalar_mul(qineg, q_i[:, c * NQ:(c + 1) * NQ], -1.0)
                                for parity in range(2):
                                    h, po = 2 * c + parity, parity * 64
                                    vsr = vst.tile([128, N // 128, HD], F32R, tag="vsr")
                                    nc.sync.dma_start(out=vsr, in_=v_dr_t[:, :, h * HD:(h + 1) * HD])
                                    vsi = vst.tile([128, N // 128, HD], F32R, tag="vsi")
                                    nc.sync.dma_start(out=vsi, in_=v_di_t[:, :, h * HD:(h + 1) * HD])
                                    qr_sl = q_r[po:po + 64, c * NQ:(c + 1) * NQ]
                                    qi_sl = q_i[po:po + 64, c * NQ:(c + 1) * NQ]
                                    qin_sl = qineg[po:po + 64, :]
                                    apr = psAt.tile([64, NQ], F32, name="apr", tag="apr", bufs=2)
                                    api = psAt.tile([64, NQ], F32, name="api", tag="api", bufs=1)
                                    sums = psSum.tile([1, NQ], F32, name="sums", tag="sums")
                                    # Chunk pairs: elementwise ops span 1024 cols.
                                    # ACT-table discipline: Squares stream (in all
                                    # tables), Sqrts batch, Exps batch -> 2 table
                                    # loads per head.
                                    m2s = []
                                    for kp in range(N // 256):
                                        srs = scp.tile([128, 2 * NQ], F32, tag="scsrs")
                                        sqw = scp.tile([128, 2 * NQ], F32, tag="scsqw")
                                        for j in range(2):
                                            kt = 2 * kp + j
                                            kr_sl = k_r[po:po + 64, c * N + kt * 128:c * N + (kt + 1) * 128]
                                            ki_sl = k_i[po:po + 64, c * N + kt * 128:c * N + (kt + 1) * 128]
                                            sr = psS.tile([128, NQ], F32, name="sr", tag="sr")
                                            _mm(nc, sr, kr_sl, qr_sl, True, False)
                                            _mm(nc, sr, ki_sl, qi_sl, False, True)
                                            si = psS.tile([128, NQ], F32, tag="si")
                                            _mm(nc, si, ki_sl, qr_sl, True, False)
                                            _mm(nc, si, kr_sl, qin_sl, False, True)
                                            nc.vector.tensor_copy(srs[:, j * NQ:(j + 1) * NQ], sr)
                                            nc.scalar.square(sqw[:, j * NQ:(j + 1) * NQ], si)
                                        nc.vector.tensor_mul(srs, srs, srs)
                                        m2 = scp.tile([128, 2 * NQ], F32, name="scm2",
                                                      tag="scm2", bufs=2)
                                        nc.gpsimd.tensor_add(m2, srs, sqw)
                                        m2s.append(m2)
                                    for kp in range(N // 256):
                                        nc.scalar.activation(m2s[kp], m2s[kp], AF.Sqrt, bias=epsa)
                                    for kp in range(N // 256):
                                        et = expp.tile([128, 2 * NQ], F32R, name="et",
                                                       tag="et", bufs=2)
                                        nc.scalar.activation(et, m2s[kp], AF.Exp, scale=1.0 / 8.0)
                                        for j in range(2):
                                            kt = 2 * kp + j
                                            et_sl = et[:, j * NQ:(j + 1) * NQ]
                                            first = kt == 0
                                            last = kt == N // 128 - 1
                                            _mm(nc, apr, vsr[:, kt, :], et_sl, first, last)
                                            _mm(nc, api, vsi[:, kt, :], et_sl, first, last)
                                            _mm(nc, sums, ones_sb, et_sl, first, last)
                                    rsum = rsp.tile([1, NQ], F32, tag="rsum")
                                    nc.vector.reciprocal(rsum, sums)
                                    rsb = rsp.tile([64, NQ], F32, tag="rsb")
                                    nc.gpsimd.partition_broadcast(rsb, rsum)
                                    for comp, ap_ps, bc, dst in (
                                            (0, apr, vbr_sb, a_dr), (1, api, vbi_sb, a_di)):
                                        ev = aevp.tile([64, NQ], F32R, name="aevt", tag="aevt")
                                        nc.vector.tensor_mul(ev, ap_ps, rsb)
                                        nc.vector.tensor_scalar_add(ev, ev, bc[:, h:h + 1])
                                        nc.sync.dma_start(out=dst[h * HD:(h + 1) * HD, :], in_=ev)

        # ---------- stage D: proj + residual --------------------------------
        with tc.tile_pool(name="dbias", bufs=1) as dbias, \
             tc.tile_pool(name="xc1", bufs=1) as xc1_p:
            pjbr_sb = dbias.tile([128, E], F32)
            nc.sync.dma_start(out=pjbr_sb, in_=pj_brr)
            pjbi_sb = dbias.tile([128, E], F32)
            nc.sync.dma_start(out=pjbi_sb, in_=pj_bri)
            m2br_sb = dbias.tile([128, E], F32)
            nc.sync.dma_start(out=m2br_sb, in_=m2_brr)
            m2bi_sb = dbias.tile([128, E], F32)
            nc.sync.dma_start(out=m2bi_sb, in_=m2_bri)
            xc1_r = xc1_p.tile([128, (NQ // 128) * E], F32)
            xc1_i = xc1_p.tile([128, (NQ // 128) * E], F32)
            with tc.tile_pool(name="atf", bufs=1) as atf, \
                 tc.tile_pool(name="pw", bufs=2) as pw, \
                 tc.tile_pool(name="xqp", bufs=2) as xqp, \
                 tc.tile_pool(name="xbp", bufs=1) as xbp, \
                 tc.tile_pool(name="psD", bufs=8, space="PSUM") as psD:
                asr_t = atf.tile([128, CE, NQ], F32R)
                nc.sync.dma_start(out=asr_t, in_=a_dr.rearrange("(c p) q -> p c q", p=128))
                asi_t = atf.tile([128, CE, NQ], F32R)
                nc.sync.dma_start(out=asi_t, in_=a_di.rearrange("(c p) q -> p c q", p=128))
                xbr_all = xbp.tile([128, (NQ // 128) * E], F32)
                xbi_all = xbp.tile([128, (NQ // 128) * E], F32)
                for t in range(NQ // 128):
                    xq_t = xqp.tile([128, 2 * E], F32, tag="xq")
                    nc.sync.dma_start(out=xq_t, in_=xp[t * 128:(t + 1) * 128, :])
                    xqc = xq_t.rearrange("p (e c) -> p e c", c=2)
                    nc.vector.tensor_add(xbr_all[:, t * E:(t + 1) * E], xqc[:, :, 0], pjbr_sb)
                    nc.vector.tensor_add(xbi_all[:, t * E:(t + 1) * E], xqc[:, :, 1], pjbi_sb)
                for f0, fw in ((0, 512), (512, 256)):
                    ps = [[psD.tile([128, fw], F32, name="pjps", tag="pjps")
                           for _ in range(2)] for _ in range(4)]
                    for ci in range(CE):
                        w3 = pw.tile([128, 3, fw], F32R, tag="pw3")
                        nc.sync.dma_start(out=w3, in_=pj_wt3[ci * 128:(ci + 1) * 128, :, f0:f0 + fw])
                        wr, wi, win = w3[:, 0, :], w3[:, 1, :], w3[:, 2, :]
                        for t in range(4):
                            lhr = asr_t[:, ci, t * 128:(t + 1) * 128]
                            lhi = asi_t[:, ci, t * 128:(t + 1) * 128]
                            _mm(nc, ps[t][0], lhr, wr, ci == 0, False)
                            _mm(nc, ps[t][0], lhi, win, False, ci == CE - 1)
                            _mm(nc, ps[t][1], lhr, wi, ci == 0, False)
                            _mm(nc, ps[t][1], lhi, wr, False, ci == CE - 1)
                    for t in range(4):
                        nc.vector.tensor_add(xc1_r[:, t * E + f0:t * E + f0 + fw],
                                             ps[t][0], xbr_all[:, t * E + f0:t * E + f0 + fw])
                        nc.vector.tensor_add(xc1_i[:, t * E + f0:t * E + f0 + fw],
                                             ps[t][1], xbi_all[:, t * E + f0:t * E + f0 + fw])

            # ---------- stage E: LN2 + transpose ----------------------------
            with tc.tile_pool(name="xn2", bufs=1) as xn2_p:
                xn2t_r = xn2_p.tile([128, CE * NQ], F32R)
                xn2t_i = xn2_p.tile([128, CE * NQ], F32R)
                # mlp weight pools open early: weight DMAs prefetch
                # during LN2 instead of gating on its pool release.
                with tc.tile_pool(name="wm1", bufs=3) as wm1, \
                     tc.tile_pool(name="wm2", bufs=3) as wm2:
                    # ht + psM1 open before LN2's pools so the first MLP
                    # matmuls (token-half split) start mid-LN2.
                    with tc.tile_pool(name="ht", bufs=1) as ht_p:
                        hT_r = ht_p.tile([128, CM * NQ], F32R)
                        hT_i = ht_p.tile([128, CM * NQ], F32R)
                        with tc.tile_pool(name="psM1", bufs=6, space="PSUM") as psM1:
                            with tc.tile_pool(name="lnE", bufs=3) as lnE, \
                                 tc.tile_pool(name="psE", bufs=1, space="PSUM") as psE:
                                for t in range(NQ // 128):
                                    layer_norm(lnE, psE,
                                               xc1_r[:, t * E:(t + 1) * E], xc1_i[:, t * E:(t + 1) * E],
                                               gb_sb["g2r"], gb_sb["g2i"], gb_sb["b2r"], gb_sb["b2i"],
                                               xn2t_r, xn2t_i, lambda c, t=t: c * NQ + t * 128, 128)

                            # ---------- stage F.1: MLP in ----------
                            for g in range(4):
                                ps = [[psM1.tile([128, NQ], F32, name="m1ps", tag="m1ps")
                                       for _ in range(2)] for _ in range(3)]
                                for ci in range(CE):
                                    w3 = wm1.tile([128, 3, 384], F32R, tag="m1w3")
                                    nc.sync.dma_start(out=w3, in_=m1_wt3[ci * 128:(ci + 1) * 128, :, g * 384:(g + 1) * 384])
                                    wr, wi, win = w3[:, 0, :], w3[:, 1, :], w3[:, 2, :]
                                    xr_sl = xn2t_r[:, ci * NQ:(ci + 1) * NQ]
                                    xi_sl = xn2t_i[:, ci * NQ:(ci + 1) * NQ]
                                    for j in range(3):
                                        wsl = slice(j * 128, (j + 1) * 128)
                                        _mm(nc, ps[j][0], wr[:, wsl], xr_sl, ci == 0, False)
                                        _mm(nc, ps[j][0], win[:, wsl], xi_sl, False, ci == CE - 1)
                                        _mm(nc, ps[j][1], wi[:, wsl], xr_sl, ci == 0, False)
                                        _mm(nc, ps[j][1], wr[:, wsl], xi_sl, False, ci == CE - 1)
                                for j in range(3):
                                    co = 3 * g + j
                                    nc.scalar.activation(hT_r[:, co * NQ:(co + 1) * NQ], ps[j][0],
                                                         AF.Gelu, bias=m1br_sb[:, co:co + 1])
                                    nc.scalar.activation(hT_i[:, co * NQ:(co + 1) * NQ], ps[j][1],
                                                         AF.Gelu, bias=m1bi_sb[:, co:co + 1])

                        with tc.tile_pool(name="outp", bufs=1) as outp, \
                             tc.tile_pool(name="fp", bufs=2) as fp, \
                             tc.tile_pool(name="psM2", bufs=8, space="PSUM") as psM2:
                            ot = [outp.tile([128, 2 * E], F32, name=f"ot{t}", tag=f"ot{t}")
                                  for t in range(4)]
                            for f0, fw in ((0, 512), (512, 256)):
                                ps = [[psM2.tile([128, fw], F32, name="m2ps", tag="m2ps")
                                       for _ in range(2)] for _ in range(4)]
                                for ck in range(CM):
                                    w3 = wm2.tile([128, 3, fw], F32R, tag="m2w3")
                                    nc.sync.dma_start(out=w3, in_=m2_wt3[ck * 128:(ck + 1) * 128, :, f0:f0 + fw])
                                    wr, wi, win = w3[:, 0, :], w3[:, 1, :], w3[:, 2, :]
                                    for t in range(4):
                                        hr_sl = hT_r[:, ck * NQ + t * 128:ck * NQ + (t + 1) * 128]
                                        hi_sl = hT_i[:, ck * NQ + t * 128:ck * NQ + (t + 1) * 128]
                                        _mm(nc, ps[t][0], hr_sl, wr, ck == 0, False)
                                        _mm(nc, ps[t][0], hi_sl, win, False, ck == CM - 1)
                                        _mm(nc, ps[t][1], hr_sl, wi, ck == 0, False)
                                        _mm(nc, ps[t][1], hi_sl, wr, False, ck == CM - 1)
                                for t in range(4):
                                    xcb_r = fp.tile([128, fw], F32, tag="xcbr")
                                    nc.vector.tensor_add(xcb_r, xc1_r[:, t * E + f0:t * E + f0 + fw],
                                                         m2br_sb[:, f0:f0 + fw])
                                    xcb_i = fp.tile([128, fw], F32, tag="xcbi")
                                    nc.vector.tensor_add(xcb_i, xc1_i[:, t * E + f0:t * E + f0 + fw],
                                                         m2bi_sb[:, f0:f0 + fw])
                                    oc = ot[t].rearrange("p (e c) -> p e c", c=2)
                                    nc.vector.tensor_add(oc[:, f0:f0 + fw, 0], ps[t][0], xcb_r)
                                    nc.vector.tensor_add(oc[:, f0:f0 + fw, 1], ps[t][1], xcb_i)
                                    if f0 == 512:
                                        # tile complete after the 2nd f chunk
                                        nc.sync.dma_start(out=out[t * 128:(t + 1) * 128, :], in_=ot[t])

    nc.compile()
    return nc


_NC = None


def _get_program():
    global _NC
    if _NC is None:
        _NC = build_program()
    return _NC


def make_in_maps(inputs):
    f = lambda a: np.ascontiguousarray(np.asarray(a, dtype=np.float32))
    x = f(inputs["x"])
    g1, b1 = f(inputs["g1"]), f(inputs["b1"])
    g2, b2 = f(inputs["g2"]), f(inputs["b2"])

    common = {}
    for nm, key in (("qkv", "qkv"), ("m1", "m1"), ("m2", "m2"), ("proj", "pj")):
        wr = f(inputs[f"{nm}_wr"]).T
        wi = f(inputs[f"{nm}_wi"]).T
        common[f"{key}_wt3"] = np.ascontiguousarray(
            np.stack([wr, wi, -wi], axis=1))
    qbr, qbi = f(inputs["qkv_br"]), f(inputs["qkv_bi"])
    common["qkv_bcr"] = np.ascontiguousarray(qbr.reshape(18, 128).T)
    common["qkv_bci"] = np.ascontiguousarray(qbi.reshape(18, 128).T)
    m1br, m1bi = f(inputs["m1_br"]), f(inputs["m1_bi"])
    common["m1_bcr"] = np.ascontiguousarray(m1br.reshape(CM, 128).T)
    common["m1_bci"] = np.ascontiguousarray(m1bi.reshape(CM, 128).T)
    common["pj_brr"] = np.ascontiguousarray(np.tile(f(inputs["proj_br"])[None, :], (128, 1)))
    common["pj_bri"] = np.ascontiguousarray(np.tile(f(inputs["proj_bi"])[None, :], (128, 1)))
    common["m2_brr"] = np.ascontiguousarray(np.tile(f(inputs["m2_br"])[None, :], (128, 1)))
    common["m2_bri"] = np.ascontiguousarray(np.tile(f(inputs["m2_bi"])[None, :], (128, 1)))
    for nm, arr in (("g1", g1), ("b1", b1), ("g2", g2), ("b2", b2)):
        common[f"{nm}r"] = np.ascontiguousarray(arr[:, 0].reshape(CE, 128).T)
        common[f"{nm}i"] = np.ascontiguousarray(arr[:, 1].reshape(CE, 128).T)
    common["ones_in"] = np.ones((128, 1), np.float32)
    common["qkv_vbr"] = np.ascontiguousarray(qbr[2 * 768:].reshape(H, 64).T)
    common["qkv_vbi"] = np.ascontiguousarray(qbi[2 * 768:].reshape(H, 64).T)

    in_maps = []
    for core in range(NCORES):
        b, half = core // 2, core % 2
        if half == 0:
            xpm = x[b]
        else:
            xpm = np.concatenate([x[b, NQ:], x[b, :NQ]], axis=0)
        in_maps.append({"xp": np.ascontiguousarray(xpm), **common})
    return in_maps


def kernel(**inputs) -> np.ndarray:
    nc = _get_program()
    in_maps = make_in_maps(inputs)
    res = run_bass_kernel_spmd(nc, in_maps, list(range(NCORES)))
    out = np.empty((B, N, 2 * E), np.float32)
    for core in range(NCORES):
        b, half = core // 2, core % 2
        out[b, half * NQ:(half + 1) * NQ, :] = res.results[core]["out"]
    return out

